# revision 17
# baseline (speedup 1.0000x reference)
"""MCR loss kernel for Trainium2 (8 NeuronCores).

Strategy:
  - Shard batch T=16 -> 2 timesteps per core (data parallel, no collectives).
  - Per core: 6 feature planes (2 timesteps x 3 maps); part A = groups 0-3
    (partition = (g, c), 128 partitions), part B = groups 4-5 packed as
    (k, g', c) where k picks a 24-input-row strip. The B tensor is
    host-prearranged partition-major so every slab lands as one full-width
    contiguous DMA.
  - ALL input DMA rides a single HWDGE ring in need order: concurrent
    queues interleave packets per SDMA engine and roughly halve HBM
    efficiency (measured 217 vs ~430 GB/s); 64-partition transfers only
    drive half the 16 engines. Full-width single-queue streaming runs at
    ~430 GB/s, so the 28.3 MB/core input streams in ~68 us.
  - 8x8 avg-pool (sum; 1/64 folded into conv weights): per 24-row slab the
    DVE XY-reduces pooled rows 0-1 directly while the Pool engine (gpsimd)
    halves w (8->4) for row 2 with ONE elementwise add, and DVE finishes.
    DVE alone (~0.96 fp32 elem/ns) cannot keep pace with the stream; the
    split keeps both engines under the 5.5 us slab cadence. Instruction
    count is kept minimal because notification traffic rides DMA engine
    15, the straggler that paces every slab-completion semaphore.
  - Reflect-pad + 3x3 conv: scalar-engine copies build a dy-replicated
    padded tile (fp32r-rounded), then 3 PE matmuls with K=(dy,ic)=96 in
    fp32r (single-pass, 2.3x faster than fp32); LeakyReLU via scalar PSUM
    copy + vector scalar_tensor_tensor max(0.2z, z).
  - Pipelining: A streams FIRST (q0..q7) -- its 4 planes need the most
    serial xrep-copy work, which then spreads over the whole stream: LO
    convs (rows 0-11) run after slab q4, HI (rows 12-23) after q7, and
    timestep 0's Gram (3 A planes) completes + ships mid-stream. B
    streams LAST (B0-B2 + the final 24 input rows as three 8-row strips
    so the last reduce is ~1 us); B's conv runs in three row-phases
    (0-11 / 12-15 / 16-23) with its xrep copies split per 6-row i-block
    so each phase is gated only on the B slabs it actually reads. The
    only work after the final strip: B FIN convs + 2 Gram chunks + the
    t1 output DMA.
  - Gram G_t = V_t V_t^T via PE transpose + fp32r matmul over six 96-px
    chunks aligned to the conv phase boundaries of both A and B.
  - Host: matrix determinant lemma
        logdet(I_576 + a V^T V) = logdet(I_96 + a V V^T)
    so only the [2,96,96] Grams leave the device; float64 Cholesky logdets
    finish the scalar loss.
"""

import sys

import numpy as np

for _p in ("/opt/trn_rl_repo", "/opt/pypackages"):
    if _p not in sys.path:
        sys.path.append(_p)

_STATE = {}

# -------- fixed problem geometry (hardcoded per harness contract) --------
B, CCH, H, W = 16, 32, 192, 192
NCORES = 8
TPC = B // NCORES          # timesteps per core = 2
OUT = 24                   # pooled spatial size
PIX = OUT * OUT            # 576
M = 96                     # feature rows (3 maps x 32 channels)
ALPHA_E = 6.0              # 576 / (96 * eps)
ALPHA_C = 18.0             # 576 / (32 * eps)

NCH = 6                    # gram chunks of 96 pixel columns each


def _build_nc():
    import concourse.bass as bass
    import concourse.tile as tile
    from concourse import bacc, mybir

    DT = mybir.dt.float32
    DTR = mybir.dt.float32r
    nc = bacc.Bacc(
        "TRN2", target_bir_lowering=False, debug=False, num_devices=NCORES
    )

    # xa[g] for g = t*3+m, g<4 : feature-map plane stacks, host-reordered.
    # xb: B-part (groups 4,5) host-prearranged partition-major
    #     xb[k*64+g'*32+c, 24q+r, w] = plane[4+g'][c, 48q+24k+r, w]
    # so every B slab DMA is a full-width contiguous transfer.
    xa = nc.declare_dram_parameter("xa", [4, CCH, H, W], DT, isOutput=False)
    xb = nc.declare_dram_parameter("xb", [128, 96, W], DT, isOutput=False)
    wt = nc.declare_dram_parameter("wt", [3, 3, 96, 32], DT, isOutput=False)
    ident = nc.declare_dram_parameter("ident", [128, 128], DT, isOutput=False)
    g_out = nc.declare_dram_parameter("g_out", [TPC, M, M], DT, isOutput=True)

    with tile.TileContext(nc) as tc:
        with (
            tc.tile_pool(name="persist", bufs=1) as persist,
            tc.tile_pool(name="slabA", bufs=4) as slabA_pool,
            tc.tile_pool(name="slabB", bufs=3) as slabB_pool,
            tc.tile_pool(name="strip", bufs=3) as strip_pool,
            tc.tile_pool(name="xrep", bufs=5) as xrep_pool,
            tc.tile_pool(name="zc", bufs=3) as zc_pool,
            tc.tile_pool(name="s1", bufs=3) as s1_pool,
            tc.tile_pool(name="vt", bufs=3) as vt_pool,
            tc.tile_pool(name="psum", bufs=3, space="PSUM") as psum_pool,
            tc.tile_pool(name="psumt", bufs=2, space="PSUM") as psumt_pool,
            tc.tile_pool(name="psumg", bufs=1, space="PSUM") as psumg_pool,
        ):
            wt_sb = persist.tile([96, 288], DT, tag="wt")
            nc.gpsimd.dma_start(
                out=wt_sb[:].rearrange("p (m x c) -> p m x c", m=3, x=3),
                in_=wt.ap().rearrange("m x p c -> p m x c"),
            )
            id_sb = persist.tile([128, 128], DT, tag="ident")
            nc.gpsimd.dma_start(out=id_sb[:], in_=ident.ap())
            # fp32r-rounded copy of the weights (PE single-pass mode needs
            # its inputs produced as float32r)
            wt_r = persist.tile([96, 288], DTR, tag="wt_r")
            nc.scalar.copy(wt_r[:], wt_sb[:])

            # pooled layouts:
            #   A: partition (g, c), g=0..3; col = y*24 + x
            #   B: partition (k, g', c) = k*64 + g'*32 + c;
            #      col = i*72 + yq*24 + x  for global y = 6i + 3k + yq
            pooledA = persist.tile([128, PIX], DT, tag="pooledA")
            pooledB = persist.tile([128, 288], DT, tag="pooledB")
            v_t0 = persist.tile([96, PIX], DT, tag="v0")
            v_t1 = persist.tile([96, PIX], DT, tag="v1")
            v_t = [v_t0, v_t1]
            g_sb = persist.tile([96, TPC * 96], DT, tag="g")

            def padd(out, in0, in1):
                nc.gpsimd.tensor_add(out, in0, in1)

            def reduce24(slab, out3):
                # 3-pooled-row slab: DVE XY-reduces rows 0-1 directly; Pool
                # halves w for row 2 (one add), DVE finishes. High priority:
                # downstream conv/STT work must never displace a reduce in
                # the engine's static order.
                s1 = s1_pool.tile([128, 768], DT, tag="s1")
                with tc.high_priority():
                    nc.vector.tensor_reduce(
                        out=out3[:, 0:2, :],
                        in_=slab[:, 0:3072].rearrange(
                            "p (y r x w) -> p y x r w", y=2, r=8, x=24, w=8
                        ),
                        axis=mybir.AxisListType.XY,
                        op=mybir.AluOpType.add,
                    )
                    v8 = slab[:, 3072:4608].rearrange(
                        "p (q w) -> p q w", q=192, w=8
                    )
                    padd(
                        s1[:].rearrange("p (q w) -> p q w", q=192, w=4),
                        v8[:, :, 0:4],
                        v8[:, :, 4:8],
                    )
                    nc.vector.tensor_reduce(
                        out=out3[:, 2:3, :],
                        in_=s1[:].rearrange(
                            "p (y r x w) -> p y x r w", y=1, r=8, x=24, w=4
                        ),
                        axis=mybir.AxisListType.XY,
                        op=mybir.AluOpType.add,
                    )

            def reduce8(slab, out2):
                # 1-pooled-row strip: DVE x cols 0-15 directly; Pool halves
                # r for cols 16-23 (the contiguous 64-elem tail of each r
                # block), DVE finishes.
                s1 = s1_pool.tile([128, 768], DT, tag="s1")
                v = slab[:].rearrange("p (r x w) -> p x r w", r=8, x=24, w=8)
                with tc.high_priority():
                    nc.vector.tensor_reduce(
                        out=out2[:, 0:16],
                        in_=v[:, 0:16],
                        axis=mybir.AxisListType.XY,
                        op=mybir.AluOpType.add,
                    )
                    vr = slab[:].rearrange("p (r f) -> p r f", r=8)[
                        :, :, 128:192
                    ]
                    padd(
                        s1[:, 0:256].rearrange("p (r f) -> p r f", r=4),
                        vr[:, 0:4],
                        vr[:, 4:8],
                    )
                    nc.vector.tensor_reduce(
                        out=out2[:, 16:24].rearrange("p (y x) -> p y x", y=1),
                        in_=s1[:, 0:256].rearrange(
                            "p (y r x w) -> p y x r w", y=1, r=4, x=8, w=8
                        ),
                        axis=mybir.AxisListType.XY,
                        op=mybir.AluOpType.add,
                    )

            # ---- the stream: A q0..q7, then B0..B2, then B's last 24 input
            # rows as three 8-row strips (tiny final reduces) ----
            for q in range(8):
                rows = slice(24 * q, 24 * q + 24)
                slabA = slabA_pool.tile([128, 24 * W], DT, tag="slabA")
                nc.sync.dma_start(
                    out=slabA[:],
                    in_=xa.ap()[:, :, rows, :].rearrange(
                        "g c h w -> (g c) (h w)"
                    ),
                )
                reduce24(
                    slabA,
                    pooledA[:, q * 72 : (q + 1) * 72].rearrange(
                        "p (y x) -> p y x", y=3
                    ),
                )
            for i in range(3):
                slabB = slabB_pool.tile([128, 24 * W], DT, tag="slabB")
                nc.sync.dma_start(
                    out=slabB[:],
                    in_=xb.ap()[:, 24 * i : 24 * i + 24, :].rearrange(
                        "p h w -> p (h w)"
                    ),
                )
                reduce24(
                    slabB,
                    pooledB[:, i * 72 : (i + 1) * 72].rearrange(
                        "p (y x) -> p y x", y=3
                    ),
                )
            for j in range(3):
                strip = strip_pool.tile([128, 8 * W], DT, tag="strip")
                nc.sync.dma_start(
                    out=strip[:],
                    in_=xb.ap()[:, 72 + 8 * j : 72 + 8 * j + 8, :].rearrange(
                        "p h w -> p (h w)"
                    ),
                )
                reduce8(strip, pooledB[:, 216 + j * 24 : 216 + (j + 1) * 24])

            # ---- conv helper: 3 dx matmuls + LeakyReLU into v_t[t],
            #      covering output rows [row0, row0+nrows) ----
            def conv_part(t, m, xr3, row0, nrows, deprio):
                ncols = nrows * 24
                pcf = psum_pool.tile([32, 288], DT, tag="convps")
                pc = pcf[:, 0:ncols]
                for dx in range(3):
                    nc.tensor.matmul(
                        pc,
                        wt_r[:, (m * 3 + dx) * 32 : (m * 3 + dx + 1) * 32],
                        xr3[:, row0 : row0 + nrows, dx : dx + 24],
                        start=(dx == 0),
                        stop=(dx == 2),
                    )
                # LeakyReLU(0.2) == max(0.2*z, z); PSUM may feed only one
                # non-scalar input, so stage a copy through SBUF first
                zcf = zc_pool.tile([32, 288], DT, tag="zcopy")
                zc = zcf[:, 0:ncols]
                nc.scalar.copy(zc, pc)
                vdst = v_t[t][
                    m * 32 : (m + 1) * 32,
                    row0 * 24 : row0 * 24 + ncols,
                ]
                # de-prioritize mid-stream STTs so the list scheduler never
                # slots them ahead of a pooling reduce in vector's order
                with tc.high_priority(offset=-1000000 if deprio else 0):
                    nc.vector.scalar_tensor_tensor(
                        out=vdst,
                        in0=zc,
                        scalar=0.2,
                        in1=pc,
                        op0=mybir.AluOpType.mult,
                        op1=mybir.AluOpType.max,
                    )

            # ---- Gram accumulators + chunk helper ----
            gps = []
            for ti in range(TPC):
                gp = psumg_pool.tile([96, 96], DT, tag=f"gram{ti}")
                gps.append(gp)

            def gram_chunk(t, ci, eng):
                lo, hi = 96 * ci, 96 * ci + 96
                vslice = v_t[t][:, lo:hi]
                pt = psumt_pool.tile([128, 96], DT, tag="vtps")
                nc.tensor.transpose(pt[:96, :], vslice, id_sb[:96, :96])
                vt = vt_pool.tile([128, 96], DTR, tag="vt")
                (nc.scalar.copy if eng == "s" else nc.vector.tensor_copy)(
                    vt[:96, :], pt[:96, :]
                )
                nc.tensor.matmul(
                    gps[t][:], vt[:96, :], vt[:96, :],
                    start=(ci == 0), stop=(ci == NCH - 1),
                )

            # ---- A convs, LO phase (rows 0-11): ready after slab q4 ----
            xr3A = {}
            srcA_of = {}
            for gi in (0, 1, 2, 3):
                t, m = divmod(gi, 3)
                xrep = xrep_pool.tile([96, 24 * 26], DTR, tag="xrep")
                xr3 = xrep[:].rearrange("p (y x) -> p y x", y=OUT)
                xr3A[gi] = xr3
                srcA = pooledA[gi * 32 : gi * 32 + 32, :].rearrange(
                    "p (y x) -> p y x", y=OUT
                )
                srcA_of[gi] = srcA
                cp = nc.scalar.copy  # scalar is idle mid-stream
                for dy in range(3):
                    dst = xr3[dy * 32 : (dy + 1) * 32]
                    y0 = 1 if dy == 0 else 0
                    cp(
                        dst[:, y0:12, 1:25],
                        srcA[:, y0 + dy - 1 : 12 + dy - 1, :],
                    )
                    if dy == 0:
                        cp(dst[:, 0:1, 1:25], srcA[:, 1:2, :])
                cp(xr3[:, 0:12, 0:1], xr3[:, 0:12, 2:3])
                cp(xr3[:, 0:12, 25:26], xr3[:, 0:12, 23:24])
                conv_part(t, m, xr3, 0, 12, True)

            # t0's first gram chunks (cols 0-287 are all-A for t0)
            for ci in (0, 1, 2):
                gram_chunk(0, ci, "s")

            # ---- A convs, HI phase (rows 12-23): ready after slab q7 ----
            for gi in (0, 1, 2, 3):
                t, m = divmod(gi, 3)
                xr3 = xr3A[gi]
                srcA = srcA_of[gi]
                cp = nc.scalar.copy
                for dy in range(3):
                    dst = xr3[dy * 32 : (dy + 1) * 32]
                    y1 = 23 if dy == 2 else 24
                    cp(
                        dst[:, 12:y1, 1:25],
                        srcA[:, 12 + dy - 1 : y1 + dy - 1, :],
                    )
                    if dy == 2:
                        cp(dst[:, 23:24, 1:25], srcA[:, 22:23, :])
                cp(xr3[:, 12:24, 0:1], xr3[:, 12:24, 2:3])
                cp(xr3[:, 12:24, 25:26], xr3[:, 12:24, 23:24])
                conv_part(t, m, xr3, 12, 12, True)

            # finish + ship t0 mid-stream (scalar HWDGE ring: a ~37 KB blip
            # interleaved into the input stream is negligible)
            for ci in (3, 4, 5):
                gram_chunk(0, ci, "s")
            nc.scalar.copy(g_sb[:, 0:96], gps[0][:])
            nc.scalar.dma_start(out=g_out[0], in_=g_sb[:, 0:96])

            # ---- B xrep builds, split per 6-row i-block so each piece is
            # gated only on the B slabs it reads (i<=1: B0-B1; i=2: B2;
            # i=3: the strips).
            # dst y = y' + 1 - dy for source row y' = 6i + 3k + yq; with
            # xr6 viewed [p, i(4), y6(6), xx(26)], dst y = 6i + (yq + off),
            # off = 3k + 1 - dy in {-1..4}: offsets 0..3 stay inside a y6
            # block; -1 / 4 spill into the neighbor block. ----
            xr3B = {}

            def b_xrep_stage(gB, isl, cp):
                # isl: slice of i-blocks to build (e.g. slice(0,2))
                xr3 = xr3B[gB]
                for dy in range(3):
                    dst6 = xr3[dy * 32 : (dy + 1) * 32].rearrange(
                        "p (i y6) x -> p i y6 x", i=4
                    )
                    for k in range(2):
                        srcB = pooledB[
                            k * 64 + gB * 32 : k * 64 + gB * 32 + 32, :
                        ].rearrange("p (i yq x) -> p i yq x", i=4, yq=3)
                        off = 3 * k + 1 - dy
                        if 0 <= off <= 3:
                            cp(
                                dst6[:, isl, off : off + 3, 1:25],
                                srcB[:, isl],
                            )
                        elif off == 4:
                            cp(
                                dst6[:, isl, 4:6, 1:25],
                                srcB[:, isl, 0:2, :],
                            )
                            # dst6[:, 1:4, 0:1] <- srcB[:, 0:3, 2:3]
                            s0, s1_ = isl.start, isl.stop
                            d0, d1 = max(1, s0 + 1), min(4, s1_ + 1)
                            if d0 < d1:
                                cp(
                                    dst6[:, d0:d1, 0:1, 1:25],
                                    srcB[:, d0 - 1 : d1 - 1, 2:3, :],
                                )
                        else:  # off == -1
                            cp(
                                dst6[:, isl, 0:2, 1:25],
                                srcB[:, isl, 1:3, :],
                            )
                            # dst6[:, 0:3, 5:6] <- srcB[:, 1:4, 0:1]
                            s0, s1_ = isl.start, isl.stop
                            d0, d1 = max(0, s0 - 1), min(3, s1_ - 1)
                            if d0 < d1:
                                cp(
                                    dst6[:, d0:d1, 5:6, 1:25],
                                    srcB[:, d0 + 1 : d1 + 1, 0:1, :],
                                )
                    # reflect rows: dy=0 -> dst y0 <- y'=1 (i=0, k=0, yq=1)
                    #               dy=2 -> dst y23 <- y'=22 (i=3, k=1, yq=1)
                    if dy == 0 and isl.start == 0:
                        cp(
                            xr3[dy * 32 : (dy + 1) * 32, 0:1, 1:25],
                            pooledB[gB * 32 : gB * 32 + 32, 24:48],
                        )
                    if dy == 2 and isl.stop == 4:
                        cp(
                            xr3[dy * 32 : (dy + 1) * 32, 23:24, 1:25],
                            pooledB[64 + gB * 32 : 64 + gB * 32 + 32, 240:264],
                        )

            for gB in range(2):
                xrep = xrep_pool.tile([96, 24 * 26], DTR, tag="xrep")
                xr3B[gB] = xrep[:].rearrange("p (y x) -> p y x", y=OUT)

            # stage 1a: i-blocks 0-1 (needs B0, B1)
            for gB in range(2):
                b_xrep_stage(gB, slice(0, 2), nc.scalar.copy)
                nc.scalar.copy(
                    xr3B[gB][:, 0:11, 0:1], xr3B[gB][:, 0:11, 2:3]
                )
                nc.scalar.copy(
                    xr3B[gB][:, 0:11, 25:26], xr3B[gB][:, 0:11, 23:24]
                )
            # stage 1b: i-block 2 (needs B2) -> B conv rows 0-15
            for gB in range(2):
                t, m = divmod(4 + gB, 3)
                b_xrep_stage(gB, slice(2, 3), nc.scalar.copy)
                nc.scalar.copy(
                    xr3B[gB][:, 11:16, 0:1], xr3B[gB][:, 11:16, 2:3]
                )
                nc.scalar.copy(
                    xr3B[gB][:, 11:16, 25:26], xr3B[gB][:, 11:16, 23:24]
                )
                conv_part(t, m, xr3B[gB], 0, 12, True)
                conv_part(t, m, xr3B[gB], 12, 4, True)
            for ci in (0, 1, 2, 3):
                gram_chunk(1, ci, "s" if ci % 2 == 0 else "v")

            # stage 2: i-block 3 (needs the strips) -> B conv rows 16-23;
            # this is the ONLY work gated by the end of the stream
            for gB, cp in ((0, nc.scalar.copy), (1, nc.vector.tensor_copy)):
                t, m = divmod(4 + gB, 3)
                b_xrep_stage(gB, slice(3, 4), cp)
                cp(xr3B[gB][:, 16:24, 0:1], xr3B[gB][:, 16:24, 2:3])
                cp(xr3B[gB][:, 16:24, 25:26], xr3B[gB][:, 16:24, 23:24])
                conv_part(t, m, xr3B[gB], 16, 8, False)
            for ci, eng in ((4, "s"), (5, "v")):
                gram_chunk(1, ci, eng)
            nc.vector.tensor_copy(g_sb[:, 96:192], gps[1][:])
            nc.sync.dma_start(out=g_out[1], in_=g_sb[:, 96:192])

    nc.finalize()
    return nc


def _get_nc():
    if "nc" not in _STATE:
        _STATE["nc"] = _build_nc()
    return _STATE["nc"]


def _prep_weights(W1, W2, W3):
    # wt[m, dx, dy*32+ic, oc] = W_m[oc, ic, dy, dx] / 64   (pool-mean folded in)
    wt = np.stack(
        [np.asarray(w, np.float64).transpose(3, 2, 1, 0).reshape(3, 96, 32)
         for w in (W1, W2, W3)]
    ) / 64.0
    return np.ascontiguousarray(wt, dtype=np.float32)


def _host_loss(G):
    G = np.asarray(G, np.float64)  # [16, 96, 96]
    T = G.shape[0]
    I96 = np.eye(M)
    Me = I96[None] + ALPHA_E * G
    ld_e = 2.0 * np.log(
        np.diagonal(np.linalg.cholesky(Me), axis1=-2, axis2=-1)
    ).sum()
    blocks = np.stack(
        [G[:, 32 * c : 32 * (c + 1), 32 * c : 32 * (c + 1)] for c in range(3)]
    )  # [3, T, 32, 32]
    Mc = np.eye(32)[None, None] + ALPHA_C * blocks
    ld_c = 2.0 * np.log(
        np.diagonal(np.linalg.cholesky(Mc), axis1=-2, axis2=-1)
    ).sum()
    loss_expd = ld_e / (2.0 * T)
    loss_comp = (32.0 / M) * ld_c / (2.0 * T)
    return np.float32(loss_expd - loss_comp)


def run_device(inputs, **kw):
    """Run the bass kernel; returns (G [16,96,96], BassKernelResults)."""
    from concourse.bass_utils import run_bass_kernel_spmd

    nc = _get_nc()
    wt = _prep_weights(inputs["W1"], inputs["W2"], inputs["W3"])
    ident = np.eye(128, dtype=np.float32)
    ms = np.asarray(inputs["ms_fea"], np.float32)
    pan = np.asarray(inputs["pan_fea"], np.float32)
    alf = np.asarray(inputs["all_fea"], np.float32)
    in_maps = []
    for i in range(NCORES):
        sl = slice(TPC * i, TPC * (i + 1))
        # x[t*3+m] = (ms,pan,alf)[m][t]
        xs = np.stack([ms[sl], pan[sl], alf[sl]], axis=1).reshape(
            TPC * 3, CCH, H, W
        )
        xa = np.ascontiguousarray(xs[0:4])
        # xb[k*64+g'*32+c, 24q+r, w] = xs[4+g'][c, 48q+24k+r, w]
        xbv = xs[4:6].reshape(2, CCH, 4, 2, 24, W)  # [g', c, q, k, r, w]
        xbv = xbv.transpose(3, 0, 1, 2, 4, 5).reshape(128, 96, W)
        in_maps.append(
            {
                "xa": xa,
                "xb": np.ascontiguousarray(xbv),
                "wt": wt,
                "ident": ident,
            }
        )
    res = run_bass_kernel_spmd(nc, in_maps, core_ids=list(range(NCORES)), **kw)
    G = np.concatenate([np.asarray(r["g_out"]) for r in res.results], axis=0)
    return G, res


def kernel(**inputs):
    G, _ = run_device(inputs)
    return _host_loss(G)


# revision 20
# speedup vs baseline: 1.0223x; 1.0223x over previous
"""MCR loss kernel for Trainium2 (8 NeuronCores).

Strategy:
  - Shard batch T=16 -> 2 timesteps per core (data parallel, no collectives).
  - Per core: 6 feature planes (2 timesteps x 3 maps); part A = groups 0-3
    (partition = (g, c), 128 partitions), part B = groups 4-5 packed as
    (k, g', c) where k picks a 24-input-row strip. The B tensor is
    host-prearranged partition-major so every slab lands as one full-width
    contiguous DMA.
  - ALL input DMA rides a single HWDGE ring in need order: concurrent
    queues interleave packets per SDMA engine and roughly halve HBM
    efficiency (measured 217 vs ~430 GB/s); 64-partition transfers only
    drive half the 16 engines. Full-width single-queue streaming runs at
    ~430 GB/s, so the 28.3 MB/core input streams in ~68 us.
  - 8x8 avg-pool (sum; 1/64 folded into conv weights): per 24-row slab the
    DVE XY-reduces pooled rows 0-1 directly while the Pool engine (gpsimd)
    halves w (8->4) for row 2 with ONE elementwise add, and DVE finishes.
    DVE alone (~0.96 fp32 elem/ns) cannot keep pace with the stream; the
    split keeps both engines under the 5.5 us slab cadence. Instruction
    count is kept minimal because notification traffic rides DMA engine
    15, the straggler that paces every slab-completion semaphore.
  - Reflect-pad + 3x3 conv: scalar-engine copies build a dy-replicated
    padded tile (fp32r-rounded), then 3 PE matmuls with K=(dy,ic)=96 in
    fp32r (single-pass, 2.3x faster than fp32); LeakyReLU via scalar PSUM
    copy + vector scalar_tensor_tensor max(0.2z, z).
  - Pipelining: A streams FIRST (q0..q7) -- its 4 planes need the most
    serial xrep-copy work, which then spreads over the whole stream: LO
    convs (rows 0-11) run after slab q4, HI (rows 12-23) after q7, and
    timestep 0's Gram (3 A planes) completes + ships mid-stream. B
    streams LAST (B0-B2 + the final 24 input rows as three 8-row strips
    so the last reduce is ~1 us); B's conv runs in three row-phases
    (0-11 / 12-15 / 16-23) with its xrep copies split per 6-row i-block
    so each phase is gated only on the B slabs it actually reads. The
    only work after the final strip: B FIN convs + 2 Gram chunks + the
    t1 output DMA.
  - Gram G_t = V_t V_t^T via PE transpose + fp32r matmul over six 96-px
    chunks aligned to the conv phase boundaries of both A and B.
  - Host: matrix determinant lemma
        logdet(I_576 + a V^T V) = logdet(I_96 + a V V^T)
    so only the [2,96,96] Grams leave the device; float64 Cholesky logdets
    finish the scalar loss.
"""

import sys

import numpy as np

for _p in ("/opt/trn_rl_repo", "/opt/pypackages"):
    if _p not in sys.path:
        sys.path.append(_p)

_STATE = {}

# -------- fixed problem geometry (hardcoded per harness contract) --------
B, CCH, H, W = 16, 32, 192, 192
NCORES = 8
TPC = B // NCORES          # timesteps per core = 2
OUT = 24                   # pooled spatial size
PIX = OUT * OUT            # 576
M = 96                     # feature rows (3 maps x 32 channels)
ALPHA_E = 6.0              # 576 / (96 * eps)
ALPHA_C = 18.0             # 576 / (32 * eps)

NCH = 6                    # gram chunks of 96 pixel columns each


def _build_nc():
    import concourse.bass as bass
    import concourse.tile as tile
    from concourse import bacc, mybir

    DT = mybir.dt.float32
    DTR = mybir.dt.float32r
    nc = bacc.Bacc(
        "TRN2", target_bir_lowering=False, debug=False, num_devices=NCORES
    )

    # xa[g] for g = t*3+m, g<4 : feature-map plane stacks, host-reordered.
    # xb: B-part (groups 4,5) host-prearranged partition-major
    #     xb[k*64+g'*32+c, 24q+r, w] = plane[4+g'][c, 48q+24k+r, w]
    # so every B slab DMA is a full-width contiguous transfer.
    xa = nc.declare_dram_parameter("xa", [4, CCH, H, W], DT, isOutput=False)
    xb = nc.declare_dram_parameter("xb", [128, 96, W], DT, isOutput=False)
    wt = nc.declare_dram_parameter("wt", [3, 3, 96, 32], DT, isOutput=False)
    ident = nc.declare_dram_parameter("ident", [128, 128], DT, isOutput=False)
    g_out = nc.declare_dram_parameter("g_out", [TPC, M, M], DT, isOutput=True)

    with tile.TileContext(nc) as tc:
        with (
            tc.tile_pool(name="persist", bufs=1) as persist,
            tc.tile_pool(name="slabA", bufs=4) as slabA_pool,
            tc.tile_pool(name="slabB", bufs=3) as slabB_pool,
            tc.tile_pool(name="strip", bufs=3) as strip_pool,
            tc.tile_pool(name="xrep", bufs=5) as xrep_pool,
            tc.tile_pool(name="s1", bufs=3) as s1_pool,
            tc.tile_pool(name="vt", bufs=3) as vt_pool,
            tc.tile_pool(name="psum", bufs=3, space="PSUM") as psum_pool,
            tc.tile_pool(name="psumt", bufs=2, space="PSUM") as psumt_pool,
            tc.tile_pool(name="psumg", bufs=1, space="PSUM") as psumg_pool,
        ):
            wt_sb = persist.tile([96, 288], DT, tag="wt")
            nc.gpsimd.dma_start(
                out=wt_sb[:].rearrange("p (m x c) -> p m x c", m=3, x=3),
                in_=wt.ap().rearrange("m x p c -> p m x c"),
            )
            id_sb = persist.tile([128, 128], DT, tag="ident")
            nc.gpsimd.dma_start(out=id_sb[:], in_=ident.ap())
            # fp32r-rounded copy of the weights (PE single-pass mode needs
            # its inputs produced as float32r)
            wt_r = persist.tile([96, 288], DTR, tag="wt_r")
            nc.scalar.copy(wt_r[:], wt_sb[:])

            # pooled layouts:
            #   A: partition (g, c), g=0..3; col = y*24 + x
            #   B: partition (k, g', c) = k*64 + g'*32 + c;
            #      col = i*72 + yq*24 + x  for global y = 6i + 3k + yq
            pooledA = persist.tile([128, PIX], DT, tag="pooledA")
            pooledB = persist.tile([128, 288], DT, tag="pooledB")
            v_t0 = persist.tile([96, PIX], DT, tag="v0")
            v_t1 = persist.tile([96, PIX], DT, tag="v1")
            v_t = [v_t0, v_t1]
            g_sb = persist.tile([96, TPC * 96], DT, tag="g")

            def padd(out, in0, in1):
                nc.gpsimd.tensor_add(out, in0, in1)

            def reduce24(slab, out3):
                # 3-pooled-row slab: DVE XY-reduces rows 0-1 directly; Pool
                # halves w for row 2 (one add), DVE finishes. High priority:
                # downstream conv/STT work must never displace a reduce in
                # the engine's static order.
                s1 = s1_pool.tile([128, 768], DT, tag="s1")
                with tc.high_priority():
                    nc.vector.tensor_reduce(
                        out=out3[:, 0:2, :],
                        in_=slab[:, 0:3072].rearrange(
                            "p (y r x w) -> p y x r w", y=2, r=8, x=24, w=8
                        ),
                        axis=mybir.AxisListType.XY,
                        op=mybir.AluOpType.add,
                    )
                    v8 = slab[:, 3072:4608].rearrange(
                        "p (q w) -> p q w", q=192, w=8
                    )
                    padd(
                        s1[:].rearrange("p (q w) -> p q w", q=192, w=4),
                        v8[:, :, 0:4],
                        v8[:, :, 4:8],
                    )
                    nc.vector.tensor_reduce(
                        out=out3[:, 2:3, :],
                        in_=s1[:].rearrange(
                            "p (y r x w) -> p y x r w", y=1, r=8, x=24, w=4
                        ),
                        axis=mybir.AxisListType.XY,
                        op=mybir.AluOpType.add,
                    )

            def reduce8(slab, out2):
                # 1-pooled-row strip: DVE x cols 0-15 directly; Pool halves
                # r for cols 16-23 (the contiguous 64-elem tail of each r
                # block), DVE finishes.
                s1 = s1_pool.tile([128, 768], DT, tag="s1")
                v = slab[:].rearrange("p (r x w) -> p x r w", r=8, x=24, w=8)
                with tc.high_priority():
                    nc.vector.tensor_reduce(
                        out=out2[:, 0:16],
                        in_=v[:, 0:16],
                        axis=mybir.AxisListType.XY,
                        op=mybir.AluOpType.add,
                    )
                    vr = slab[:].rearrange("p (r f) -> p r f", r=8)[
                        :, :, 128:192
                    ]
                    padd(
                        s1[:, 0:256].rearrange("p (r f) -> p r f", r=4),
                        vr[:, 0:4],
                        vr[:, 4:8],
                    )
                    nc.vector.tensor_reduce(
                        out=out2[:, 16:24].rearrange("p (y x) -> p y x", y=1),
                        in_=s1[:, 0:256].rearrange(
                            "p (y r x w) -> p y x r w", y=1, r=4, x=8, w=8
                        ),
                        axis=mybir.AxisListType.XY,
                        op=mybir.AluOpType.add,
                    )

            # ---- the stream: A q0..q7, then B0..B2, then B's last 24 input
            # rows as three 8-row strips (tiny final reduces) ----
            for q in range(8):
                rows = slice(24 * q, 24 * q + 24)
                slabA = slabA_pool.tile([128, 24 * W], DT, tag="slabA")
                nc.sync.dma_start(
                    out=slabA[:],
                    in_=xa.ap()[:, :, rows, :].rearrange(
                        "g c h w -> (g c) (h w)"
                    ),
                )
                reduce24(
                    slabA,
                    pooledA[:, q * 72 : (q + 1) * 72].rearrange(
                        "p (y x) -> p y x", y=3
                    ),
                )
            for i in range(3):
                slabB = slabB_pool.tile([128, 24 * W], DT, tag="slabB")
                nc.sync.dma_start(
                    out=slabB[:],
                    in_=xb.ap()[:, 24 * i : 24 * i + 24, :].rearrange(
                        "p h w -> p (h w)"
                    ),
                )
                reduce24(
                    slabB,
                    pooledB[:, i * 72 : (i + 1) * 72].rearrange(
                        "p (y x) -> p y x", y=3
                    ),
                )
            for j in range(3):
                strip = strip_pool.tile([128, 8 * W], DT, tag="strip")
                nc.sync.dma_start(
                    out=strip[:],
                    in_=xb.ap()[:, 72 + 8 * j : 72 + 8 * j + 8, :].rearrange(
                        "p h w -> p (h w)"
                    ),
                )
                reduce8(strip, pooledB[:, 216 + j * 24 : 216 + (j + 1) * 24])

            # ---- conv helper: 3 dx matmuls + LeakyReLU into v_t[t],
            #      covering output rows [row0, row0+nrows). The LeakyReLU
            #      runs as ONE Act-engine activation (Lrelu, alpha=0.2)
            #      straight from PSUM -- no SBUF staging copy and, vitally,
            #      NOTHING on the vector engine: DVE's static order stays
            #      pure pooling reduces, so the list scheduler can never
            #      slot conv work ahead of a reduce. ----
            def conv_part(t, m, xr3, row0, nrows, deprio):
                ncols = nrows * 24
                pcf = psum_pool.tile([32, 288], DT, tag="convps")
                pc = pcf[:, 0:ncols]
                for dx in range(3):
                    nc.tensor.matmul(
                        pc,
                        wt_r[:, (m * 3 + dx) * 32 : (m * 3 + dx + 1) * 32],
                        xr3[:, row0 : row0 + nrows, dx : dx + 24],
                        start=(dx == 0),
                        stop=(dx == 2),
                    )
                vdst = v_t[t][
                    m * 32 : (m + 1) * 32,
                    row0 * 24 : row0 * 24 + ncols,
                ]
                nc.scalar.activation(
                    vdst, pc, mybir.ActivationFunctionType.Prelu, alpha=0.2
                )

            # ---- Gram accumulators + chunk helper ----
            gps = []
            for ti in range(TPC):
                gp = psumg_pool.tile([96, 96], DT, tag=f"gram{ti}")
                gps.append(gp)

            def gram_chunk(t, ci, eng):
                lo, hi = 96 * ci, 96 * ci + 96
                vslice = v_t[t][:, lo:hi]
                pt = psumt_pool.tile([128, 96], DT, tag="vtps")
                nc.tensor.transpose(pt[:96, :], vslice, id_sb[:96, :96])
                vt = vt_pool.tile([128, 96], DTR, tag="vt")
                (nc.scalar.copy if eng == "s" else nc.vector.tensor_copy)(
                    vt[:96, :], pt[:96, :]
                )
                nc.tensor.matmul(
                    gps[t][:], vt[:96, :], vt[:96, :],
                    start=(ci == 0), stop=(ci == NCH - 1),
                )

            # ---- A convs, LO phase (rows 0-11): ready after slab q4 ----
            xr3A = {}
            srcA_of = {}
            for gi in (0, 1, 2, 3):
                t, m = divmod(gi, 3)
                xrep = xrep_pool.tile([96, 24 * 26], DTR, tag="xrep")
                xr3 = xrep[:].rearrange("p (y x) -> p y x", y=OUT)
                xr3A[gi] = xr3
                srcA = pooledA[gi * 32 : gi * 32 + 32, :].rearrange(
                    "p (y x) -> p y x", y=OUT
                )
                srcA_of[gi] = srcA
                cp = nc.scalar.copy  # scalar is idle mid-stream
                for dy in range(3):
                    dst = xr3[dy * 32 : (dy + 1) * 32]
                    y0 = 1 if dy == 0 else 0
                    cp(
                        dst[:, y0:12, 1:25],
                        srcA[:, y0 + dy - 1 : 12 + dy - 1, :],
                    )
                    if dy == 0:
                        cp(dst[:, 0:1, 1:25], srcA[:, 1:2, :])
                cp(xr3[:, 0:12, 0:1], xr3[:, 0:12, 2:3])
                cp(xr3[:, 0:12, 25:26], xr3[:, 0:12, 23:24])
                conv_part(t, m, xr3, 0, 12, True)

            # t0's first gram chunks (cols 0-287 are all-A for t0)
            for ci in (0, 1, 2):
                gram_chunk(0, ci, "s")

            # ---- A convs, HI phase (rows 12-23): ready after slab q7 ----
            for gi in (0, 1, 2, 3):
                t, m = divmod(gi, 3)
                xr3 = xr3A[gi]
                srcA = srcA_of[gi]
                cp = nc.scalar.copy
                for dy in range(3):
                    dst = xr3[dy * 32 : (dy + 1) * 32]
                    y1 = 23 if dy == 2 else 24
                    cp(
                        dst[:, 12:y1, 1:25],
                        srcA[:, 12 + dy - 1 : y1 + dy - 1, :],
                    )
                    if dy == 2:
                        cp(dst[:, 23:24, 1:25], srcA[:, 22:23, :])
                cp(xr3[:, 12:24, 0:1], xr3[:, 12:24, 2:3])
                cp(xr3[:, 12:24, 25:26], xr3[:, 12:24, 23:24])
                conv_part(t, m, xr3, 12, 12, True)

            # finish + ship t0 mid-stream (scalar HWDGE ring: a ~37 KB blip
            # interleaved into the input stream is negligible)
            for ci in (3, 4, 5):
                gram_chunk(0, ci, "s")
            nc.scalar.copy(g_sb[:, 0:96], gps[0][:])
            nc.scalar.dma_start(out=g_out[0], in_=g_sb[:, 0:96])

            # ---- B xrep builds, split per 6-row i-block so each piece is
            # gated only on the B slabs it reads (i<=1: B0-B1; i=2: B2;
            # i=3: the strips).
            # dst y = y' + 1 - dy for source row y' = 6i + 3k + yq; with
            # xr6 viewed [p, i(4), y6(6), xx(26)], dst y = 6i + (yq + off),
            # off = 3k + 1 - dy in {-1..4}: offsets 0..3 stay inside a y6
            # block; -1 / 4 spill into the neighbor block. ----
            xr3B = {}

            def b_xrep_stage(gB, isl, cp):
                # isl: slice of i-blocks to build (e.g. slice(0,2))
                xr3 = xr3B[gB]
                for dy in range(3):
                    dst6 = xr3[dy * 32 : (dy + 1) * 32].rearrange(
                        "p (i y6) x -> p i y6 x", i=4
                    )
                    for k in range(2):
                        srcB = pooledB[
                            k * 64 + gB * 32 : k * 64 + gB * 32 + 32, :
                        ].rearrange("p (i yq x) -> p i yq x", i=4, yq=3)
                        off = 3 * k + 1 - dy
                        if 0 <= off <= 3:
                            cp(
                                dst6[:, isl, off : off + 3, 1:25],
                                srcB[:, isl],
                            )
                        elif off == 4:
                            cp(
                                dst6[:, isl, 4:6, 1:25],
                                srcB[:, isl, 0:2, :],
                            )
                            # dst6[:, 1:4, 0:1] <- srcB[:, 0:3, 2:3]
                            s0, s1_ = isl.start, isl.stop
                            d0, d1 = max(1, s0 + 1), min(4, s1_ + 1)
                            if d0 < d1:
                                cp(
                                    dst6[:, d0:d1, 0:1, 1:25],
                                    srcB[:, d0 - 1 : d1 - 1, 2:3, :],
                                )
                        else:  # off == -1
                            cp(
                                dst6[:, isl, 0:2, 1:25],
                                srcB[:, isl, 1:3, :],
                            )
                            # dst6[:, 0:3, 5:6] <- srcB[:, 1:4, 0:1]
                            s0, s1_ = isl.start, isl.stop
                            d0, d1 = max(0, s0 - 1), min(3, s1_ - 1)
                            if d0 < d1:
                                cp(
                                    dst6[:, d0:d1, 5:6, 1:25],
                                    srcB[:, d0 + 1 : d1 + 1, 0:1, :],
                                )
                    # reflect rows: dy=0 -> dst y0 <- y'=1 (i=0, k=0, yq=1)
                    #               dy=2 -> dst y23 <- y'=22 (i=3, k=1, yq=1)
                    if dy == 0 and isl.start == 0:
                        cp(
                            xr3[dy * 32 : (dy + 1) * 32, 0:1, 1:25],
                            pooledB[gB * 32 : gB * 32 + 32, 24:48],
                        )
                    if dy == 2 and isl.stop == 4:
                        cp(
                            xr3[dy * 32 : (dy + 1) * 32, 23:24, 1:25],
                            pooledB[64 + gB * 32 : 64 + gB * 32 + 32, 240:264],
                        )

            for gB in range(2):
                xrep = xrep_pool.tile([96, 24 * 26], DTR, tag="xrep")
                xr3B[gB] = xrep[:].rearrange("p (y x) -> p y x", y=OUT)

            # stage 1a: i-blocks 0-1 (needs B0, B1)
            for gB in range(2):
                b_xrep_stage(gB, slice(0, 2), nc.scalar.copy)
                nc.scalar.copy(
                    xr3B[gB][:, 0:11, 0:1], xr3B[gB][:, 0:11, 2:3]
                )
                nc.scalar.copy(
                    xr3B[gB][:, 0:11, 25:26], xr3B[gB][:, 0:11, 23:24]
                )
            # stage 1b: i-block 2 (needs B2) -> B conv rows 0-15
            for gB in range(2):
                t, m = divmod(4 + gB, 3)
                b_xrep_stage(gB, slice(2, 3), nc.scalar.copy)
                nc.scalar.copy(
                    xr3B[gB][:, 11:16, 0:1], xr3B[gB][:, 11:16, 2:3]
                )
                nc.scalar.copy(
                    xr3B[gB][:, 11:16, 25:26], xr3B[gB][:, 11:16, 23:24]
                )
                conv_part(t, m, xr3B[gB], 0, 12, True)
                conv_part(t, m, xr3B[gB], 12, 4, True)
            for ci in (0, 1, 2, 3):
                gram_chunk(1, ci, "s" if ci % 2 == 0 else "v")

            # stage 2: i-block 3 (needs the strips) -> B conv rows 16-23;
            # this is the ONLY work gated by the end of the stream
            for gB, cp in ((0, nc.scalar.copy), (1, nc.vector.tensor_copy)):
                t, m = divmod(4 + gB, 3)
                b_xrep_stage(gB, slice(3, 4), cp)
                cp(xr3B[gB][:, 16:24, 0:1], xr3B[gB][:, 16:24, 2:3])
                cp(xr3B[gB][:, 16:24, 25:26], xr3B[gB][:, 16:24, 23:24])
                conv_part(t, m, xr3B[gB], 16, 8, False)
            for ci, eng in ((4, "s"), (5, "v")):
                gram_chunk(1, ci, eng)
            nc.vector.tensor_copy(g_sb[:, 96:192], gps[1][:])
            nc.sync.dma_start(out=g_out[1], in_=g_sb[:, 96:192])

    nc.finalize()
    return nc


def _get_nc():
    if "nc" not in _STATE:
        _STATE["nc"] = _build_nc()
    return _STATE["nc"]


def _prep_weights(W1, W2, W3):
    # wt[m, dx, dy*32+ic, oc] = W_m[oc, ic, dy, dx] / 64   (pool-mean folded in)
    wt = np.stack(
        [np.asarray(w, np.float64).transpose(3, 2, 1, 0).reshape(3, 96, 32)
         for w in (W1, W2, W3)]
    ) / 64.0
    return np.ascontiguousarray(wt, dtype=np.float32)


def _host_loss(G):
    G = np.asarray(G, np.float64)  # [16, 96, 96]
    T = G.shape[0]
    I96 = np.eye(M)
    Me = I96[None] + ALPHA_E * G
    ld_e = 2.0 * np.log(
        np.diagonal(np.linalg.cholesky(Me), axis1=-2, axis2=-1)
    ).sum()
    blocks = np.stack(
        [G[:, 32 * c : 32 * (c + 1), 32 * c : 32 * (c + 1)] for c in range(3)]
    )  # [3, T, 32, 32]
    Mc = np.eye(32)[None, None] + ALPHA_C * blocks
    ld_c = 2.0 * np.log(
        np.diagonal(np.linalg.cholesky(Mc), axis1=-2, axis2=-1)
    ).sum()
    loss_expd = ld_e / (2.0 * T)
    loss_comp = (32.0 / M) * ld_c / (2.0 * T)
    return np.float32(loss_expd - loss_comp)


def run_device(inputs, **kw):
    """Run the bass kernel; returns (G [16,96,96], BassKernelResults)."""
    from concourse.bass_utils import run_bass_kernel_spmd

    nc = _get_nc()
    wt = _prep_weights(inputs["W1"], inputs["W2"], inputs["W3"])
    ident = np.eye(128, dtype=np.float32)
    ms = np.asarray(inputs["ms_fea"], np.float32)
    pan = np.asarray(inputs["pan_fea"], np.float32)
    alf = np.asarray(inputs["all_fea"], np.float32)
    in_maps = []
    for i in range(NCORES):
        sl = slice(TPC * i, TPC * (i + 1))
        # x[t*3+m] = (ms,pan,alf)[m][t]
        xs = np.stack([ms[sl], pan[sl], alf[sl]], axis=1).reshape(
            TPC * 3, CCH, H, W
        )
        xa = np.ascontiguousarray(xs[0:4])
        # xb[k*64+g'*32+c, 24q+r, w] = xs[4+g'][c, 48q+24k+r, w]
        xbv = xs[4:6].reshape(2, CCH, 4, 2, 24, W)  # [g', c, q, k, r, w]
        xbv = xbv.transpose(3, 0, 1, 2, 4, 5).reshape(128, 96, W)
        in_maps.append(
            {
                "xa": xa,
                "xb": np.ascontiguousarray(xbv),
                "wt": wt,
                "ident": ident,
            }
        )
    res = run_bass_kernel_spmd(nc, in_maps, core_ids=list(range(NCORES)), **kw)
    G = np.concatenate([np.asarray(r["g_out"]) for r in res.results], axis=0)
    return G, res


def kernel(**inputs):
    G, _ = run_device(inputs)
    return _host_loss(G)


# revision 23
# speedup vs baseline: 1.1422x; 1.1173x over previous
"""MCR loss kernel for Trainium2 (8 NeuronCores).

Strategy:
  - Shard batch T=16 -> 2 timesteps per core (data parallel, no collectives).
  - Per core: 6 feature planes (2 timesteps x 3 maps); part A = groups 0-3
    (partition = (g, c), 128 partitions), part B = groups 4-5 packed as
    (k, g', c) where k picks a 24-input-row strip. The B tensor is
    host-prearranged partition-major so every slab lands as one full-width
    contiguous DMA.
  - ALL input DMA rides a single HWDGE ring in need order: concurrent
    queues interleave packets per SDMA engine and roughly halve HBM
    efficiency (measured 217 vs ~430 GB/s); 64-partition transfers only
    drive half the 16 engines. Full-width single-queue streaming runs at
    ~430 GB/s, so the 28.3 MB/core input streams in ~68 us.
  - 8x8 avg-pool (sum; 1/64 folded into conv weights) as vector-engine XY
    reduces SPLIT 2:1 with the Pool engine (gpsimd): DVE reduces fp32 at
    ~0.96 elem/ns/partition and alone (55k elem/partition = 58 us) cannot
    keep pace with the 67 us stream; Pool runs the same reduce at ~0.5
    elem/ns (Q7 software, 0.42 eff), so a per-slab 2-row/1-row split keeps
    both engines at ~35 us and the DMA ring gapless. Reduces are emitted
    at high priority so conv work never displaces them.
  - Reflect-pad + 3x3 conv: scalar-engine copies build a dy-replicated
    padded tile (fp32r-rounded), then 3 PE matmuls with K=(dy,ic)=96 in
    fp32r (single-pass, 2.3x faster than fp32); LeakyReLU via scalar PSUM
    copy + vector scalar_tensor_tensor max(0.2z, z).
  - Pipelining: B is streamed first and fully processed mid-stream. The
    A convs run in FOUR output-row phases so nearly all conv/Gram work
    overlaps the stream and only a tiny tail trails the last slab:
      LO rows 0-11 (ready after A slab q4), MID rows 12-18 (after q6),
      FIN1 rows 19-21 (after the 8-row slab of pooled row 22),
      FIN2 rows 22-23 (after the last 8-row slab, pooled row 23).
  - Gram G_t = V_t V_t^T via PE transpose + fp32r matmul over 7 pixel
    chunks aligned to the phase column ranges (96/96/96 | 96/72 | 72 | 48)
    so each chunk fires as soon as its conv phase completes.
  - Host: matrix determinant lemma
        logdet(I_576 + a V^T V) = logdet(I_96 + a V V^T)
    so only the [2,96,96] Grams leave the device; float64 Cholesky logdets
    finish the scalar loss.
"""

import sys

import numpy as np

for _p in ("/opt/trn_rl_repo", "/opt/pypackages"):
    if _p not in sys.path:
        sys.path.append(_p)

_STATE = {}

# -------- fixed problem geometry (hardcoded per harness contract) --------
B, CCH, H, W = 16, 32, 192, 192
NCORES = 8
TPC = B // NCORES          # timesteps per core = 2
OUT = 24                   # pooled spatial size
PIX = OUT * OUT            # 576
M = 96                     # feature rows (3 maps x 32 channels)
ALPHA_E = 6.0              # 576 / (96 * eps)
ALPHA_C = 18.0             # 576 / (32 * eps)

# A-conv output-row phases and the gram pixel chunks gated by each phase.
# Phase p covers conv output rows [r0, r1); gram chunk (lo, hi) contracts
# V columns [lo, hi) and needs all conv phases with r1*24 <= hi done.
PHASES = ((0, 12), (12, 19), (19, 22), (22, 24))
CHUNKS = (
    (0, 96), (96, 192), (192, 288),      # LO   (cols 0..288)
    (288, 384), (384, 456),              # MID  (cols 288..456)
    (456, 528),                          # FIN1 (cols 456..528)
    (528, 576),                          # FIN2 (cols 528..576)
)
NCH = len(CHUNKS)


def _build_nc():
    import concourse.bass as bass
    import concourse.tile as tile
    from concourse import bacc, mybir

    DT = mybir.dt.float32
    DTR = mybir.dt.float32r
    nc = bacc.Bacc(
        "TRN2", target_bir_lowering=False, debug=False, num_devices=NCORES
    )

    # xa[g] for g = t*3+m, g<4 : feature-map plane stacks, host-reordered.
    # xb: B-part (groups 4,5) host-prearranged partition-major
    #     xb[k*64+g'*32+c, 24q+r, w] = plane[4+g'][c, 48q+24k+r, w]
    # so every B slab DMA is a full-width contiguous transfer.
    xa = nc.declare_dram_parameter("xa", [4, CCH, H, W], DT, isOutput=False)
    xb = nc.declare_dram_parameter("xb", [128, 96, W], DT, isOutput=False)
    wt = nc.declare_dram_parameter("wt", [3, 3, 96, 32], DT, isOutput=False)
    ident = nc.declare_dram_parameter("ident", [128, 128], DT, isOutput=False)
    g_out = nc.declare_dram_parameter("g_out", [TPC, M, M], DT, isOutput=True)

    with tile.TileContext(nc) as tc:
        with (
            tc.tile_pool(name="persist", bufs=1) as persist,
            tc.tile_pool(name="slabA", bufs=5) as slabA_pool,
            tc.tile_pool(name="slabA8", bufs=3) as slabA8_pool,
            tc.tile_pool(name="slabB", bufs=3) as slabB_pool,
            tc.tile_pool(name="xrep", bufs=5) as xrep_pool,
            tc.tile_pool(name="s1", bufs=3) as s1_pool,
            tc.tile_pool(name="vt", bufs=3) as vt_pool,
            tc.tile_pool(name="psum", bufs=3, space="PSUM") as psum_pool,
            tc.tile_pool(name="psumt", bufs=2, space="PSUM") as psumt_pool,
            tc.tile_pool(name="psumg", bufs=1, space="PSUM") as psumg_pool,
        ):
            wt_sb = persist.tile([96, 288], DT, tag="wt")
            nc.gpsimd.dma_start(
                out=wt_sb[:].rearrange("p (m x c) -> p m x c", m=3, x=3),
                in_=wt.ap().rearrange("m x p c -> p m x c"),
            )
            id_sb = persist.tile([128, 128], DT, tag="ident")
            nc.gpsimd.dma_start(out=id_sb[:], in_=ident.ap())
            # fp32r-rounded copy of the weights (PE single-pass mode needs
            # its inputs produced as float32r)
            wt_r = persist.tile([96, 288], DTR, tag="wt_r")
            nc.scalar.copy(wt_r[:], wt_sb[:])

            # pooled layouts:
            #   A: partition (g, c), g=0..3; col = y*24 + x
            #   B: partition (k, g', c) = k*64 + g'*32 + c;
            #      col = i*72 + yq*24 + x  for global y = 6i + 3k + yq
            pooledA = persist.tile([128, PIX], DT, tag="pooledA")
            pooledB = persist.tile([128, 288], DT, tag="pooledB")
            v_t0 = persist.tile([96, PIX], DT, tag="v0")
            v_t1 = persist.tile([96, PIX], DT, tag="v1")
            v_t = [v_t0, v_t1]
            g_sb = persist.tile([96, TPC * 96], DT, tag="g")

            def padd(out, in0, in1):
                nc.gpsimd.tensor_add(out, in0, in1)

            def reduce24(slab, out3):
                # 3-pooled-row slab: DVE XY-reduces rows 0-1 directly; for
                # row 2 the Pool engine (gpsimd) first halves w (8->4, one
                # elementwise add -- Pool has no free-axis tensor_reduce and
                # runs adds at only ~0.4 elem/ns), then DVE finishes with a
                # small XY reduce. This takes ~1.6us/slab off DVE (which
                # alone cannot keep pace with the stream) at ONE extra Pool
                # instruction per slab -- instruction count matters because
                # notification traffic rides DMA engine 15, the straggler
                # that paces every slab-completion semaphore.
                # High priority: downstream conv/STT work must never
                # displace a reduce in the engine's static order.
                s1 = s1_pool.tile([128, 768], DT, tag="s1")
                with tc.high_priority():
                    nc.vector.tensor_reduce(
                        out=out3[:, 0:2, :],
                        in_=slab[:, 0:3072].rearrange(
                            "p (y r x w) -> p y x r w", y=2, r=8, x=24, w=8
                        ),
                        axis=mybir.AxisListType.XY,
                        op=mybir.AluOpType.add,
                    )
                    v8 = slab[:, 3072:4608].rearrange(
                        "p (q w) -> p q w", q=192, w=8
                    )
                    padd(
                        s1[:].rearrange("p (q w) -> p q w", q=192, w=4),
                        v8[:, :, 0:4],
                        v8[:, :, 4:8],
                    )
                    nc.vector.tensor_reduce(
                        out=out3[:, 2:3, :],
                        in_=s1[:].rearrange(
                            "p (y r x w) -> p y x r w", y=1, r=8, x=24, w=4
                        ),
                        axis=mybir.AxisListType.XY,
                        op=mybir.AluOpType.add,
                    )

            def reduce8(slab, out2, last=False):
                # 1-pooled-row slab: DVE x cols 0-15 directly; Pool halves
                # r for cols 16-23 (the contiguous 64-elem tail of each r
                # block), DVE finishes. The LAST slab's reduces are pushed
                # late in the static order so vector drains ready FIN1 work
                # instead of idling at a semaphore for the final slab.
                s1 = s1_pool.tile([128, 768], DT, tag="s1")
                v = slab[:].rearrange("p (r x w) -> p x r w", r=8, x=24, w=8)
                with tc.high_priority(offset=-2_000_000 if last else None):
                    nc.vector.tensor_reduce(
                        out=out2[:, 0:16],
                        in_=v[:, 0:16],
                        axis=mybir.AxisListType.XY,
                        op=mybir.AluOpType.add,
                    )
                    vr = slab[:].rearrange("p (r f) -> p r f", r=8)[
                        :, :, 128:192
                    ]
                    padd(
                        s1[:, 0:256].rearrange("p (r f) -> p r f", r=4),
                        vr[:, 0:4],
                        vr[:, 4:8],
                    )
                    nc.vector.tensor_reduce(
                        out=out2[:, 16:24].rearrange("p (y x) -> p y x", y=1),
                        in_=s1[:, 0:256].rearrange(
                            "p (y r x w) -> p y x r w", y=1, r=4, x=8, w=8
                        ),
                        axis=mybir.AxisListType.XY,
                        op=mybir.AluOpType.add,
                    )

            # ---- pooling. ALL input DMA on the single sync HWDGE ring in
            # need order: multiple concurrent queues interleave packets and
            # halve HBM efficiency (measured 217 vs 424 GB/s).
            #   B slab i covers input rows 48i..48i+47 as two 24-row k
            #   strips; A = seven 24-row slabs + three 8-row slabs (small
            #   final reduces shorten the post-stream tail). ----
            def dma_B(i):
                # one full-width contiguous DMA (xb host-prearranged)
                slabB = slabB_pool.tile([128, 24 * W], DT, tag="slabB")
                nc.sync.dma_start(
                    out=slabB[:],
                    in_=xb.ap()[:, 24 * i : 24 * i + 24, :].rearrange(
                        "p h w -> p (h w)"
                    ),
                )
                reduce24(
                    slabB,
                    pooledB[:, i * 72 : (i + 1) * 72].rearrange(
                        "p (y x) -> p y x", y=3
                    ),
                )

            def dma_A(q):
                rows = slice(24 * q, 24 * q + 24)
                slabA = slabA_pool.tile([128, 24 * W], DT, tag="slabA")
                nc.sync.dma_start(
                    out=slabA[:],
                    in_=xa.ap()[:, :, rows, :].rearrange(
                        "g c h w -> (g c) (h w)"
                    ),
                )
                reduce24(
                    slabA,
                    pooledA[:, q * 72 : (q + 1) * 72].rearrange(
                        "p (y x) -> p y x", y=3
                    ),
                )

            # B first: all of B lands by ~1/3 of the stream, so the B conv
            # path completes mid-stream and the tail is A-only.
            # A: q0..q6 are 24-row slabs; the final 24 input rows stream as
            # three 8-row slabs so the last reduce is ~1us, not ~3us.
            for i in range(4):
                dma_B(i)
            for j in range(7):
                dma_A(j)
            for yrow in (21, 22, 23):
                rows = slice(8 * yrow, 8 * yrow + 8)
                slabA8 = slabA8_pool.tile([128, 8 * W], DT, tag="slabA8")
                nc.sync.dma_start(
                    out=slabA8[:],
                    in_=xa.ap()[:, :, rows, :].rearrange(
                        "g c h w -> (g c) (h w)"
                    ),
                )
                reduce8(
                    slabA8,
                    pooledA[:, yrow * 24 : (yrow + 1) * 24],
                    last=(yrow == 23),
                )

            # ---- conv helper: 3 dx matmuls + LeakyReLU into v_t[t],
            #      covering output rows [row0, row0+nrows) ----
            def conv_part(t, m, xr3, row0, nrows, deprio):
                ncols = nrows * 24
                pcf = psum_pool.tile([32, 288], DT, tag="convps")
                pc = pcf[:, 0:ncols]
                for dx in range(3):
                    nc.tensor.matmul(
                        pc,
                        wt_r[:, (m * 3 + dx) * 32 : (m * 3 + dx + 1) * 32],
                        xr3[:, row0 : row0 + nrows, dx : dx + 24],
                        start=(dx == 0),
                        stop=(dx == 2),
                    )
                vdst = v_t[t][
                    m * 32 : (m + 1) * 32,
                    row0 * 24 : row0 * 24 + ncols,
                ]
                # LeakyReLU(0.2) as ONE Act-engine Prelu straight from PSUM
                # (verified exact: Prelu honors immediate alpha; Lrelu is
                # hardwired to 0.01). No SBUF staging copy and nothing on
                # the vector engine, whose static order stays pure reduces.
                nc.scalar.activation(
                    vdst, pc, mybir.ActivationFunctionType.Prelu, alpha=0.2
                )

            # ---- B-group convs (gi = 4, 5): processed first, mid-stream ----
            # xrep rows: dst y = y' + 1 - dy for source row y' = 6i + 3k + yq.
            # With xr6 = xrep viewed [p, yb(4), y6(6), xx(26)], dst y =
            # 6i + (yq + off), off = 3k + 1 - dy in {-1..4}: offsets 0..3 stay
            # inside a y6 block (one copy); -1 / 4 split into two copies.
            for gB in range(2):
                t, m = divmod(4 + gB, 3)
                xrep = xrep_pool.tile([96, 24 * 26], DTR, tag="xrep")
                xr3 = xrep[:].rearrange("p (y x) -> p y x", y=OUT)
                for dy in range(3):
                    dst6 = xr3[dy * 32 : (dy + 1) * 32].rearrange(
                        "p (i y6) x -> p i y6 x", i=4
                    )
                    for k in range(2):
                        srcB = pooledB[
                            k * 64 + gB * 32 : k * 64 + gB * 32 + 32, :
                        ].rearrange("p (i yq x) -> p i yq x", i=4, yq=3)
                        off = 3 * k + 1 - dy
                        if 0 <= off <= 3:
                            nc.scalar.copy(
                                dst6[:, :, off : off + 3, 1:25], srcB[:]
                            )
                        elif off == 4:
                            nc.scalar.copy(
                                dst6[:, :, 4:6, 1:25], srcB[:, :, 0:2, :]
                            )
                            nc.scalar.copy(
                                dst6[:, 1:4, 0:1, 1:25], srcB[:, 0:3, 2:3, :]
                            )
                        else:  # off == -1
                            nc.scalar.copy(
                                dst6[:, :, 0:2, 1:25], srcB[:, :, 1:3, :]
                            )
                            nc.scalar.copy(
                                dst6[:, 0:3, 5:6, 1:25], srcB[:, 1:4, 0:1, :]
                            )
                    # reflect rows: dy=0 -> dst y0 <- y'=1 (k=0, i=0, yq=1);
                    #               dy=2 -> dst y23 <- y'=22 (k=1, i=3, yq=1)
                    if dy == 0:
                        nc.scalar.copy(
                            xr3[dy * 32 : (dy + 1) * 32, 0:1, 1:25],
                            pooledB[gB * 32 : gB * 32 + 32, 24:48],
                        )
                    if dy == 2:
                        nc.scalar.copy(
                            xr3[dy * 32 : (dy + 1) * 32, 23:24, 1:25],
                            pooledB[64 + gB * 32 : 64 + gB * 32 + 32, 240:264],
                        )
                nc.scalar.copy(xr3[:, :, 0:1], xr3[:, :, 2:3])
                nc.scalar.copy(xr3[:, :, 25:26], xr3[:, :, 23:24])
                for half in range(2):
                    conv_part(t, m, xr3, 12 * half, 12, True)

            # ---- A-group convs, LO phase (output rows 0..11): needs only
            #      pooled y' <= 12 (slabs q0..q4), so this and the gram
            #      chunks 0-2 run DURING the A stream (also keeps PE warm) ----
            xr3A = {}
            srcA_of = {}
            for gi in (3, 0, 1, 2):
                t, m = divmod(gi, 3)
                xrep = xrep_pool.tile([96, 24 * 26], DTR, tag="xrep")
                xr3 = xrep[:].rearrange("p (y x) -> p y x", y=OUT)
                xr3A[gi] = xr3
                srcA = pooledA[gi * 32 : gi * 32 + 32, :].rearrange(
                    "p (y x) -> p y x", y=OUT
                )
                srcA_of[gi] = srcA
                cp = nc.scalar.copy  # scalar is idle mid-stream
                for dy in range(3):
                    dst = xr3[dy * 32 : (dy + 1) * 32]
                    y0 = 1 if dy == 0 else 0
                    cp(
                        dst[:, y0:12, 1:25],
                        srcA[:, y0 + dy - 1 : 12 + dy - 1, :],
                    )
                    if dy == 0:
                        cp(dst[:, 0:1, 1:25], srcA[:, 1:2, :])
                cp(xr3[:, 0:12, 0:1], xr3[:, 0:12, 2:3])
                cp(xr3[:, 0:12, 25:26], xr3[:, 0:12, 23:24])
                conv_part(t, m, xr3, 0, 12, True)

            # ---- Gram accumulators + chunk helper ----
            gps = []
            for ti in range(TPC):
                gp = psumg_pool.tile([96, 96], DT, tag=f"gram{ti}")
                gps.append(gp)

            def gram_chunk(t, ci, eng):
                lo, hi = CHUNKS[ci]
                sz = hi - lo
                vslice = v_t[t][:, lo:hi]
                pt = psumt_pool.tile([128, 96], DT, tag="vtps")
                nc.tensor.transpose(pt[:sz, :], vslice, id_sb[:96, :96])
                vt = vt_pool.tile([128, 96], DTR, tag="vt")
                (nc.scalar.copy if eng == "s" else nc.vector.tensor_copy)(
                    vt[:sz, :], pt[:sz, :]
                )
                nc.tensor.matmul(
                    gps[t][:], vt[:sz, :], vt[:sz, :],
                    start=(ci == 0), stop=(ci == NCH - 1),
                )

            # Gram chunks 0-2 (conv LO columns): mid-stream, CAST on scalar
            for ci in (0, 1, 2):
                for t in (1, 0):
                    gram_chunk(t, ci, "s")

            # ---- A-group convs, MID phase (rows 12..18): needs pooled
            #      y' <= 20, i.e. after slab q6 -- still mid-stream ----
            def a_phase(gi, r0, r1, cp):
                t, m = divmod(gi, 3)
                xr3 = xr3A[gi]
                srcA = srcA_of[gi]
                for dy in range(3):
                    dst = xr3[dy * 32 : (dy + 1) * 32]
                    y1 = min(r1, 23) if dy == 2 else r1
                    cp(
                        dst[:, r0:y1, 1:25],
                        srcA[:, r0 + dy - 1 : y1 + dy - 1, :],
                    )
                    if dy == 2 and y1 < r1:
                        # reflect: dst row 23 <- pooled row 22
                        cp(dst[:, 23:24, 1:25], srcA[:, 22:23, :])
                cp(xr3[:, r0:r1, 0:1], xr3[:, r0:r1, 2:3])
                cp(xr3[:, r0:r1, 25:26], xr3[:, r0:r1, 23:24])
                conv_part(t, m, xr3, r0, r1 - r0, True)

            for gi in (3, 0, 1, 2):
                a_phase(gi, 12, 19, nc.scalar.copy)
            for ci in (3, 4):
                for t in (1, 0):
                    gram_chunk(t, ci, "v")

            # ---- FIN1 (rows 19..21): after the 8-row slab of pooled row 22;
            #      copies split across scalar and vector ----
            for gi in (3, 0, 1, 2):
                cp = nc.vector.tensor_copy if gi in (3, 1) else nc.scalar.copy
                a_phase(gi, 19, 22, cp)
            for t, eng in ((1, "s"), (0, "v")):
                gram_chunk(t, 5, eng)

            # ---- FIN2 (rows 22..23): the only work gated by the last slab ----
            for gi in (3, 0, 1, 2):
                cp = nc.vector.tensor_copy if gi in (3, 1) else nc.scalar.copy
                a_phase(gi, 22, 24, cp)
            for t, eng in ((1, "s"), (0, "v")):
                gram_chunk(t, 6, eng)

            for t, ring, cpf in (
                (1, nc.gpsimd, nc.scalar.copy),
                (0, nc.sync, nc.vector.tensor_copy),
            ):
                cpf(g_sb[:, t * 96 : (t + 1) * 96], gps[t][:])
                ring.dma_start(
                    out=g_out[t], in_=g_sb[:, t * 96 : (t + 1) * 96]
                )

    nc.finalize()
    return nc


def _get_nc():
    if "nc" not in _STATE:
        _STATE["nc"] = _build_nc()
    return _STATE["nc"]


def _prep_weights(W1, W2, W3):
    # wt[m, dx, dy*32+ic, oc] = W_m[oc, ic, dy, dx] / 64   (pool-mean folded in)
    wt = np.stack(
        [np.asarray(w, np.float64).transpose(3, 2, 1, 0).reshape(3, 96, 32)
         for w in (W1, W2, W3)]
    ) / 64.0
    return np.ascontiguousarray(wt, dtype=np.float32)


def _host_loss(G):
    G = np.asarray(G, np.float64)  # [16, 96, 96]
    T = G.shape[0]
    I96 = np.eye(M)
    Me = I96[None] + ALPHA_E * G
    ld_e = 2.0 * np.log(
        np.diagonal(np.linalg.cholesky(Me), axis1=-2, axis2=-1)
    ).sum()
    blocks = np.stack(
        [G[:, 32 * c : 32 * (c + 1), 32 * c : 32 * (c + 1)] for c in range(3)]
    )  # [3, T, 32, 32]
    Mc = np.eye(32)[None, None] + ALPHA_C * blocks
    ld_c = 2.0 * np.log(
        np.diagonal(np.linalg.cholesky(Mc), axis1=-2, axis2=-1)
    ).sum()
    loss_expd = ld_e / (2.0 * T)
    loss_comp = (32.0 / M) * ld_c / (2.0 * T)
    return np.float32(loss_expd - loss_comp)


def run_device(inputs, **kw):
    """Run the bass kernel; returns (G [16,96,96], BassKernelResults)."""
    from concourse.bass_utils import run_bass_kernel_spmd

    nc = _get_nc()
    wt = _prep_weights(inputs["W1"], inputs["W2"], inputs["W3"])
    ident = np.eye(128, dtype=np.float32)
    ms = np.asarray(inputs["ms_fea"], np.float32)
    pan = np.asarray(inputs["pan_fea"], np.float32)
    alf = np.asarray(inputs["all_fea"], np.float32)
    in_maps = []
    for i in range(NCORES):
        sl = slice(TPC * i, TPC * (i + 1))
        # x[t*3+m] = (ms,pan,alf)[m][t]
        xs = np.stack([ms[sl], pan[sl], alf[sl]], axis=1).reshape(
            TPC * 3, CCH, H, W
        )
        xa = np.ascontiguousarray(xs[0:4])
        # xb[k*64+g'*32+c, 24q+r, w] = xs[4+g'][c, 48q+24k+r, w]
        xbv = xs[4:6].reshape(2, CCH, 4, 2, 24, W)  # [g', c, q, k, r, w]
        xbv = xbv.transpose(3, 0, 1, 2, 4, 5).reshape(128, 96, W)
        in_maps.append(
            {
                "xa": xa,
                "xb": np.ascontiguousarray(xbv),
                "wt": wt,
                "ident": ident,
            }
        )
    res = run_bass_kernel_spmd(nc, in_maps, core_ids=list(range(NCORES)), **kw)
    G = np.concatenate([np.asarray(r["g_out"]) for r in res.results], axis=0)
    return G, res


def kernel(**inputs):
    G, _ = run_device(inputs)
    return _host_loss(G)
